# revision 33
# baseline (speedup 1.0000x reference)
"""Swin BasicLayer (depth=2 windowed attention) Trainium2 kernel.

Sharding: data-parallel over batch B=8 across 8 NeuronCores; weights
replicated. Each core runs both depths (regular + shifted windows) over
its [12544, 384] image in [C, token] layout.

v10: fp8(e4m3)-DoubleRow QKV projections (channel-pairs packed on
partitions; x and V/QK weights quantized to fp8 with power-of-2
scaling, rescale folded into the exp scale and the vt ones column),
bf16 attention + output projection, f32 psum accumulate everywhere.
exp(S+E) computed as exp(S)*exp(E) so QK needs no psum bias preload.
Software-pipelined group loop: V-proj 2 groups ahead, normalize 1
group behind, transposes 2 groups behind.
"""
import numpy as np
import ml_dtypes

import concourse.bass as bass
import concourse.tile as tile
from concourse import bacc, mybir
from concourse.bass_utils import run_bass_kernel_spmd

f32 = mybir.dt.float32
bf16 = mybir.dt.bfloat16
f8 = mybir.dt.float8e4
AF = mybir.ActivationFunctionType
ALU = mybir.AluOpType
PM = mybir.MatmulPerfMode

B, H, W, C = 8, 112, 112, 384
NH, HD, WS = 12, 32, 7
N = WS * WS          # 49
L = H * W            # 12544
NBAND = H // WS      # 16
BAND = WS * W        # 784
HALF = BAND // 2     # 392
NWB = W // WS        # 16 windows per band
NG = NWB // 2        # 8 window-pair groups
DEPTH = 2

SCW = 16.0           # fp8 weight scale
SCX = [1.0, 32.0]    # fp8 x scale per depth (xmid stored *32)


def _rel_pos_index():
    coords = np.stack(np.meshgrid(np.arange(WS), np.arange(WS), indexing='ij')).reshape(2, -1)
    rel = (coords[:, :, None] - coords[:, None, :]).transpose(1, 2, 0).copy()
    rel[..., 0] += WS - 1
    rel[..., 1] += WS - 1
    rel[..., 0] *= 2 * WS - 1
    return rel.sum(-1)


def _masks():
    """The 4 distinct [N, N] shifted-window masks by type 2*(i==15)+(j==15)."""
    ws, shift = WS, WS // 2
    img = np.zeros((H, W), dtype=np.float32)
    slices = (slice(0, -ws), slice(-ws, -shift), slice(-shift, None))
    cnt = 0
    for hs in slices:
        for wsl in slices:
            img[hs, wsl] = cnt
            cnt += 1
    mw = img.reshape(H // ws, ws, W // ws, ws).transpose(0, 2, 1, 3).reshape(-1, ws * ws)
    diff = mw[:, None, :] - mw[:, :, None]
    mask = np.where(diff != 0, -100.0, 0.0).astype(np.float32)  # [nW, N, N]
    nwr = H // ws
    m = {}
    for ti, widx in ((0, 0), (1, nwr - 1), (2, (nwr - 1) * nwr), (3, nwr * nwr - 1)):
        m[ti] = mask[widx]
    return m


def _band_ranges(i, shift):
    r0 = (WS * i + shift) % H
    n0 = min(WS, H - r0)
    rows = [(0, r0, n0)]
    if n0 < WS:
        rows.append((n0, 0, WS - n0))
    if shift == 0:
        cols = [(0, 0, W)]
    else:
        cols = [(0, shift, W - shift), (W - shift, 0, shift)]
    return rows, cols


def _build(nbands=NBAND, ndepth=DEPTH):
    nc = bacc.Bacc("TRN2", target_bir_lowering=False, debug=False, num_devices=8)

    xin = nc.dram_tensor("xin", [C, L], f8, kind="ExternalInput")
    xmid = nc.dram_tensor("xmid", [C, L], f8)
    xout = nc.dram_tensor("xout", [C, L], f32, kind="ExternalOutput")
    dwqk8p = nc.dram_tensor("wqk8p", [DEPTH, 128, 4 * C], f8, kind="ExternalInput")
    dwqk8u = nc.dram_tensor("wqk8u", [DEPTH, 128, 2 * C], f8, kind="ExternalInput")
    dwv8p = nc.dram_tensor("wv8p", [DEPTH, 128, 2 * C], f8, kind="ExternalInput")
    dwv8u = nc.dram_tensor("wv8u", [DEPTH, 128, C], f8, kind="ExternalInput")
    dwp = nc.dram_tensor("wp", [DEPTH, C, C], bf16, kind="ExternalInput")
    dbqk = nc.dram_tensor("bqk", [DEPTH, 2 * C], f32, kind="ExternalInput")
    dbp = nc.dram_tensor("bp", [DEPTH, C], f32, kind="ExternalInput")
    de0 = nc.dram_tensor("e0", [128, 12 * N], bf16, kind="ExternalInput")
    de1 = nc.dram_tensor("e1", [128, 4 * 12 * N], bf16, kind="ExternalInput")
    didn = nc.dram_tensor("idn", [128, 128], bf16, kind="ExternalInput")

    def evac_copy(use_dve, out_ap, in_ap):
        if use_dve:
            nc.vector.tensor_copy(out_ap, in_ap)
        else:
            nc.scalar.activation(out_ap, in_ap, AF.Identity, bias=0.0)

    def evac_bias(use_dve, out_ap, in_ap, bias_ap, scale=1.0):
        if use_dve:
            if scale == 1.0:
                nc.vector.tensor_scalar_add(out_ap, in_ap, bias_ap)
            else:
                nc.vector.tensor_scalar(out_ap, in_ap, scale, bias_ap,
                                        ALU.mult, ALU.add)
        else:
            nc.scalar.activation(out_ap, in_ap, AF.Identity, bias=bias_ap,
                                 scale=scale)

    with tile.TileContext(nc) as tc:
        cpool = tc.alloc_tile_pool(name="const", bufs=1)
        p_xr = tc.alloc_tile_pool(name="xr", bufs=5)
        p_xw = tc.alloc_tile_pool(name="xw", bufs=4)
        p_qkb = tc.alloc_tile_pool(name="qkb", bufs=14)
        p_outT = tc.alloc_tile_pool(name="outT", bufs=3)
        p_yr = tc.alloc_tile_pool(name="yr", bufs=4)
        p_pt = tc.alloc_tile_pool(name="pt", bufs=4)
        p_tm = tc.alloc_tile_pool(name="ptm", bufs=3)
        p_vt = tc.alloc_tile_pool(name="vt", bufs=8)
        p_on = tc.alloc_tile_pool(name="onat", bufs=4)
        p_rt = tc.alloc_tile_pool(name="rt", bufs=3)
        ps_proj = tc.alloc_tile_pool(name="psproj", bufs=2, space="PSUM")
        ps_s = tc.alloc_tile_pool(name="pss", bufs=1, space="PSUM")
        ps_av = tc.alloc_tile_pool(name="psav", bufs=2, space="PSUM")

        # constants
        wqk8p = [cpool.tile([128, 4 * C], f8, tag=f"wqk8p{d}", name=f"wqk8p{d}")
                 for d in range(DEPTH)]
        wqk8u = [cpool.tile([128, 2 * C], f8, tag=f"wqk8u{d}", name=f"wqk8u{d}")
                 for d in range(DEPTH)]
        wv8p = [cpool.tile([128, 2 * C], f8, tag=f"wv8p{d}", name=f"wv8p{d}")
                for d in range(DEPTH)]
        wv8u = [cpool.tile([128, C], f8, tag=f"wv8u{d}", name=f"wv8u{d}")
                for d in range(DEPTH)]
        wp = [[cpool.tile([128, C], bf16, tag=f"wp{d}{cc}", name=f"wp{d}{cc}") for cc in range(3)]
              for d in range(DEPTH)]
        bqk = [cpool.tile([128, 6], f32, tag=f"bqk{d}", name=f"bqk{d}") for d in range(DEPTH)]
        bp = [cpool.tile([128, 3], f32, tag=f"bp{d}", name=f"bp{d}") for d in range(DEPTH)]
        e0 = cpool.tile([128, 12 * N], bf16, tag="e0", name="e0")
        e1 = cpool.tile([128, 4 * 12 * N], bf16, tag="e1", name="e1")
        idn = cpool.tile([128, 128], bf16, tag="idn", name="idn")
        for d in range(DEPTH):
            nc.sync.dma_start(wqk8p[d][:], dwqk8p[d])
            nc.sync.dma_start(wqk8u[d][:], dwqk8u[d])
            nc.sync.dma_start(wv8p[d][:], dwv8p[d])
            nc.sync.dma_start(wv8u[d][:], dwv8u[d])
            for cc in range(3):
                nc.sync.dma_start(wp[d][cc][:], dwp[d, cc * 128:(cc + 1) * 128, :])
            nc.sync.dma_start(bqk[d][:], dbqk[d].rearrange("(o p) -> p o", p=128))
            nc.sync.dma_start(bp[d][:], dbp[d].rearrange("(o p) -> p o", p=128))
        nc.sync.dma_start(e0[:], de0[:])
        nc.sync.dma_start(e1[:], de1[:])
        nc.sync.dma_start(idn[:], didn[:])

        for d in range(ndepth):
            shift = 0 if d == 0 else WS // 2
            xsrc = xin if d == 0 else xmid
            if d == ndepth - 1:
                ydst, ydt = xout, f32
            else:
                ydst, ydt = xmid, f8
            escale = 1.0 / (SCW * SCX[d]) ** 2
            ones_v = SCW * SCX[d]
            wqk8p_v = wqk8p[d][:].rearrange("p (i o) -> p i o", i=2)
            wv8p_v = wv8p[d][:].rearrange("p (i o) -> p i o", i=2)
            XR, XW_, QKB = {}, {}, {}

            def band_dma(bj):
                # load band: channel-pair-packed fp8 (chans 0..255 on
                # partitions as pairs) + plain fp8 (chans 256..383)
                rws, cls = _band_ranges(bj, shift)
                xrp = p_xr.tile([128, 2 * BAND], f8, tag="xrp", name="xrp")
                xru = p_xr.tile([128, BAND], f8, tag="xru", name="xru")
                xsa = xsrc[:, :]
                for (dr, sr, nr) in rws:
                    for (dc, sc, ncl) in cls:
                        for i in range(2):
                            dst = xrp[:, i * BAND:(i + 1) * BAND].rearrange(
                                "p (r c) -> p r c", r=WS)[:, dr:dr + nr, dc:dc + ncl]
                            src = bass.AP(xsa.tensor, xsa.offset + i * L + sr * W + sc,
                                          [[2 * L, 128], [W, nr], [1, ncl]])
                            nc.sync.dma_start(dst, src)
                        dstu = xru[:].rearrange("p (r c) -> p r c",
                                                r=WS)[:, dr:dr + nr, dc:dc + ncl]
                        srcu = bass.AP(xsa.tensor, xsa.offset + 256 * L + sr * W + sc,
                                       [[L, 128], [W, nr], [1, ncl]])
                        nc.sync.dma_start(dstu, srcu)
                XR[bj] = (xrp, xru)

            def band_relayout(bj):
                # window-major relayout (raster -> 49w+7r+j), fp8
                xrp, xru = XR.pop(bj)
                xwp = p_xw.tile([128, 2 * BAND], f8, tag="xwp", name="xwp")
                xwu = p_xw.tile([128, BAND], f8, tag="xwu", name="xwu")
                for i in range(2):
                    in_p = xrp[:, i * BAND:(i + 1) * BAND].rearrange(
                        "p (r w j) -> p w r j", r=WS, w=NWB, j=WS)
                    out_p = xwp[:, i * BAND:(i + 1) * BAND].rearrange(
                        "p (w r j) -> p w r j", w=NWB, r=WS, j=WS)
                    evac_copy((bj + i) % 2, out_p, in_p)
                in_u = xru[:].rearrange("p (r w j) -> p w r j", r=WS, w=NWB, j=WS)
                out_u = xwu[:].rearrange("p (w r j) -> p w r j", w=NWB, r=WS, j=WS)
                evac_copy(bj % 2, out_u, in_u)
                XW_[bj] = (xwp, xwu)

            def qkb_chain(bj, idx):
                # one (half, oc) q/k projection chain for band bj
                # (fp8 DoubleRow pass + plain fp8 pass)
                half, oc = idx // 6, idx % 6
                if idx == 0:
                    QKB[bj] = [p_qkb.tile([128, BAND], bf16, tag="qkb", name="qkb")
                               for _ in range(6)]
                xwp, xwu = XW_[bj]
                xwp_v = xwp[:].rearrange("p (i t) -> p i t", i=2)
                qkb = QKB[bj]
                ps = ps_proj.tile([128, HALF], f32, tag="psproj", name="psproj", padded_shape=[128, 512])
                nc.tensor.matmul(
                    ps[:], wqk8p_v[:, :, oc * 128:(oc + 1) * 128],
                    xwp_v[:, :, half * HALF:(half + 1) * HALF],
                    start=True, stop=False, perf_mode=PM.DoubleRow)
                nc.tensor.matmul(
                    ps[:], wqk8u[d][:, oc * 128:(oc + 1) * 128],
                    xwu[:, half * HALF:(half + 1) * HALF],
                    start=False, stop=True)
                evac_bias((oc + half) % 2,
                          qkb[oc][:, half * HALF:(half + 1) * HALF],
                          ps[:], bqk[d][:, oc:oc + 1])

            for bi in range(nbands):
                rows, cols = _band_ranges(bi, shift)
                if bi == 0:
                    band_dma(0)
                    band_relayout(0)
                    for idx in range(12):
                        qkb_chain(0, idx)
                if bi + 1 < nbands:
                    band_dma(bi + 1)
                qkb = QKB.pop(bi)
                xwp, xwu = XW_[bi]
                xwp_v = xwp[:].rearrange("p (i t) -> p i t", i=2)
                # ---- attention groups, software-pipelined
                outT = p_outT.tile([128, 3 * BAND], bf16, tag="outT", name="outT")
                vts = [None] * NG
                vps = [None] * NG

                def do_vproj_mm(g):
                    vp = ps_proj.tile([128, C], f32, tag="psproj", name="psproj", padded_shape=[128, 512])
                    nc.tensor.matmul(vp[0:98, :],
                                     xwp_v[:, :, 98 * g:98 * g + 98],
                                     wv8p_v[:],
                                     start=True, stop=False, perf_mode=PM.DoubleRow)
                    nc.tensor.matmul(vp[0:98, :],
                                     xwu[:, 98 * g:98 * g + 98],
                                     wv8u[d][:],
                                     start=False, stop=True)
                    vps[g] = vp

                def do_vproj_evac(g):
                    vp = vps[g]
                    vt = p_vt.tile([128, 2 * 12 * 33], bf16, tag="vt", name="vt")
                    iv = vp[0:98, :].rearrange("p (h e) -> p h e", h=12)
                    ov = vt[0:98, 0:396].rearrange("p (h e) -> p h e", e=33)[:, :, 0:32]
                    nc.vector.tensor_copy(ov, iv)
                    nc.vector.memset(
                        vt[:, 0:396].rearrange("p (h e) -> p h e", e=33)[:, :, 32:33],
                        ones_v)
                    nc.sync.dma_start(vt[64:113, 396:792], vt[49:98, 0:396])
                    vts[g] = vt

                def ebase_sel(g):
                    if d == 0:
                        return e0, 0
                    if g < NG - 1:
                        return e1, (0 if bi < NBAND - 1 else 1) * 12 * N
                    return e1, (2 if bi < NBAND - 1 else 3) * 12 * N

                def do_transpose(g, on):
                    # transpose out [n, c] -> [c, n]; [0:113, 128] per cc
                    # (rows 49:64 are junk, skipped by the evac)
                    tp = ps_proj.tile([128, 342], bf16, tag="psproj", name="pstp", padded_shape=[128, 512])
                    for cc in range(3):
                        nc.tensor.transpose(
                            tp[:, 114 * cc:114 * cc + 113],
                            on[0:113, cc * 128:(cc + 1) * 128],
                            idn[0:113, 0:113])
                    oT = outT[:].rearrange("p f -> p f")
                    out_ap = bass.AP(oT.tensor, oT.offset + 98 * g,
                                     [oT.ap[0], [BAND, 3], [N, 2], [1, N]])
                    tpa = tp[:].rearrange("p f -> p f")
                    in_ap = bass.AP(tpa.tensor, tpa.offset,
                                    [tpa.ap[0], [114, 3], [64, 2], [1, N]])
                    evac_copy(g % 2, out_ap, in_ap)

                pts = [None] * NG
                ons = [None] * NG

                def do_exp(g, sp):
                    # exp(escale*S^T) -> bf16 tmp at 52-pitch (scalar), then
                    # pt = tmp * exp(E) elementwise (vector). exp(E) is
                    # host-precomputed; masked slots are exact zeros.
                    tm = p_tm.tile([128, 12 * 52], bf16, tag="ptm", name="ptm")
                    pt = p_pt.tile([128, 12 * 52], bf16, tag="pt", name="pt")
                    spa = sp[:]
                    tm_ = tm[:].rearrange("p f -> p f")
                    po_ = pt[:].rearrange("p f -> p f")
                    exp_in = bass.AP(spa.tensor, spa.offset,
                                     [spa.ap[0], [512, 4], [N, 3], [1, N]])
                    exp_out = bass.AP(tm_.tensor, tm_.offset,
                                      [tm_.ap[0], [52, 4], [4 * 52, 3], [1, N]])
                    nc.scalar.activation(exp_out, exp_in, AF.Exp, scale=escale)
                    et, ebase = ebase_sel(g)
                    ea = et[:].rearrange("p f -> p f")
                    e_in = bass.AP(ea.tensor, ea.offset + ebase,
                                   [ea.ap[0], [N, 4], [4 * N, 3], [1, N]])
                    t_in = bass.AP(tm_.tensor, tm_.offset,
                                   [tm_.ap[0], [52, 4], [4 * 52, 3], [1, N]])
                    p_out = bass.AP(po_.tensor, po_.offset,
                                    [po_.ap[0], [52, 4], [4 * 52, 3], [1, N]])
                    nc.vector.tensor_tensor(p_out, t_in, e_in, ALU.mult)
                    pts[g] = pt

                avs = [None] * NG

                def do_av_mm(g):
                    # AV (+ fused row-sums via ones column of vt); h-outer so
                    # consecutive matmuls alternate PE tiles (LDW pull-ahead)
                    pt, vt = pts[g], vts[g]
                    av = ps_av.tile([128, 12 * 33], f32, tag="psav", name="psav", padded_shape=[128, 512])
                    for h in range(NH):
                        for s in range(2):
                            nc.tensor.matmul(
                                av[64 * s:64 * s + 49, 33 * h:33 * h + 33],
                                pt[64 * s:64 * s + 49, 52 * h:52 * h + N],
                                vt[64 * s:64 * s + 49,
                                   396 * s + 33 * h:396 * s + 33 * h + 33],
                                start=True, stop=True,
                                tile_position=(64 * s, 64 * s))
                    avs[g] = av

                def do_norm(g):
                    av = avs[g]
                    rt = p_rt.tile([128, 12], f32, tag="rt", name="rt")
                    nc.vector.reciprocal(
                        rt[:].rearrange("p (h e) -> p h e", e=1),
                        av[:].rearrange("p (h e) -> p h e", e=33)[:, :, 32:33])
                    on = p_on.tile([128, C], bf16, tag="onat", name="onat")
                    rap = rt[:]
                    rbc = bass.AP(rap.tensor, rap.offset, [rap.ap[0], [1, 12], [0, 32]])
                    nc.vector.tensor_tensor(
                        on[:].rearrange("p (h e) -> p h e", e=32),
                        av[:].rearrange("p (h e) -> p h e", e=33)[:, :, 0:32],
                        rbc, ALU.mult)
                    ons[g] = on

                do_vproj_mm(0)
                do_vproj_evac(0)
                do_vproj_mm(1)
                do_vproj_evac(1)
                for g in range(NG):
                    sp = ps_s.tile([128, 2048], f32, tag="pss", name="pss")
                    for s in range(2):
                        w = 2 * g + s
                        for h in range(NH):
                            po = 32 * (h % 4)
                            kT = qkb[3 + h // 4][po:po + 32, N * w:N * w + N]
                            qT = qkb[h // 4][po:po + 32, N * w:N * w + N]
                            out = sp[64 * s:64 * s + 49,
                                     512 * (h % 4) + N * (h // 4):
                                     512 * (h % 4) + N * (h // 4) + N]
                            nc.tensor.matmul(out, kT, qT, start=True, stop=True,
                                             tile_position=(po, 64 * s))
                    do_exp(g, sp)
                    if bi + 1 < nbands:
                        if g == 0:
                            band_relayout(bi + 1)
                        elif g >= 2:
                            qkb_chain(bi + 1, 2 * (g - 2))
                            qkb_chain(bi + 1, 2 * (g - 2) + 1)
                    if g + 2 < NG:
                        do_vproj_mm(g + 2)
                    if g >= 2:
                        do_transpose(g - 2, ons[g - 2])
                    do_av_mm(g)
                    if g >= 1:
                        do_norm(g - 1)
                    if g + 2 < NG:
                        do_vproj_evac(g + 2)
                do_norm(NG - 1)
                do_transpose(NG - 2, ons[NG - 2])
                do_transpose(NG - 1, ons[NG - 1])
                # ---- output projection (+ window-major -> raster relayout)
                yr = [p_yr.tile([128, BAND], ydt, tag="yr", name="yr") for _ in range(3)]
                ysc = SCX[1] if d == 0 and ndepth > 1 else 1.0
                for half in range(2):
                    for oc in range(3):
                        ps = ps_proj.tile([128, HALF], f32, tag="psproj", name="psproj", padded_shape=[128, 512])
                        for cc in range(3):
                            nc.tensor.matmul(
                                ps[:], wp[d][cc][:, oc * 128:(oc + 1) * 128],
                                outT[:, cc * BAND + half * HALF:
                                     cc * BAND + (half + 1) * HALF],
                                start=(cc == 0), stop=(cc == 2))
                        in_ap = ps[:].rearrange("p (w r j) -> p w r j",
                                                w=NG, r=WS, j=WS)
                        out_ap = yr[oc][:].rearrange(
                            "p (r w j) -> p w r j", r=WS, w=NWB,
                            j=WS)[:, NG * half:NG * half + NG, :, :]
                        evac_bias((oc + half) % 2, out_ap, in_ap,
                                  bp[d][:, oc:oc + 1], scale=ysc)
                # ---- store band
                for oc in range(3):
                    yr3 = yr[oc][:].rearrange("p (r c) -> p r c", r=WS)
                    dst3 = ydst[oc * 128:(oc + 1) * 128, :].rearrange(
                        "p (r c) -> p r c", r=H)
                    for (dr, sr, nr) in rows:
                        for (dc, sc, ncl) in cols:
                            nc.sync.dma_start(dst3[:, sr:sr + nr, sc:sc + ncl],
                                              yr3[:, dr:dr + nr, dc:dc + ncl])
                del XW_[bi]
            if d == 0 and ndepth > 1:
                tc.strict_bb_all_engine_barrier()

        for p in (ps_av, ps_s, ps_proj, p_rt, p_on, p_vt,
                  p_tm, p_pt, p_yr, p_outT, p_qkb, p_xw, p_xr, cpool):
            p.release()

    nc.compile()
    return nc


_NC = None


def _get_nc():
    global _NC
    if _NC is None:
        _NC = _build()
    return _NC


def _bf16(x):
    return np.ascontiguousarray(x).astype(ml_dtypes.bfloat16)


def _f8(x):
    return np.ascontiguousarray(x).astype(mybir.dt.np(f8))


def _pair_pack(w):
    """[256, M] -> [128, 2M], i-outer: col i*M+m = w[2p+i, m]."""
    M = w.shape[1]
    return w.reshape(128, 2 * M)


def _host_prep(qkv_w, qkv_b, proj_w, proj_b, rpb_table):
    scale = HD ** -0.5
    rpi = _rel_pos_index()
    masks = _masks()
    common = {}
    wqk8p = np.zeros((DEPTH, 128, 4 * C), np.float32)
    wqk8u = np.zeros((DEPTH, 128, 2 * C), np.float32)
    wv8p = np.zeros((DEPTH, 128, 2 * C), np.float32)
    wv8u = np.zeros((DEPTH, 128, C), np.float32)
    wpp = np.zeros((DEPTH, C, C), np.float32)
    bqk = np.zeros((DEPTH, 2 * C), np.float32)
    bpp = np.zeros((DEPTH, C), np.float32)
    for d in range(DEPTH):
        wq = qkv_w[d][:2 * C].T.copy()        # [C, 2C] (q then k)
        wq[:, :C] *= scale
        wq *= SCW
        wqk8p[d] = _pair_pack(wq[:256])
        wqk8u[d] = wq[256:]
        wvv = qkv_w[d][2 * C:].T * SCW        # [C, C]
        wv8p[d] = _pair_pack(wvv[:256])
        wv8u[d] = wvv[256:]
        wpp[d] = proj_w[d].T
        bq = qkv_b[d][:2 * C].copy()
        bq[:C] *= scale
        bqk[d] = bq * (SCW * SCX[d])
        bv = qkv_b[d][2 * C:]
        bpp[d] = proj_b[d] + proj_w[d] @ bv
    bpp[0] *= SCX[1]   # depth-0 output stored as fp8 * SCX[1]
    common["wqk8p"] = _f8(wqk8p)
    common["wqk8u"] = _f8(wqk8u)
    common["wv8p"] = _f8(wv8p)
    common["wv8u"] = _f8(wv8u)
    common["wp"] = _bf16(wpp)
    common["bqk"] = bqk
    common["bp"] = bpp

    # E tiles: rows 0-48 -> m, rows 64-112 -> m-64; value exp(bias[h,n,m]+mask[n,m])
    def etile(d, type_a, type_b):
        bias = rpb_table[d][rpi]              # [N, N, NH]
        t = np.zeros((128, 12 * N), np.float32)
        for s, ty in ((0, type_a), (1, type_b)):
            bm = bias + (masks[ty][:, :, None] if ty is not None else 0.0)
            ev = bm.transpose(2, 1, 0)   # [NH, m, n] (log domain)
            blk = ev.transpose(1, 0, 2).reshape(N, 12 * N)  # row m, col h*N+n
            t[64 * s:64 * s + N, :] = blk
        return t

    common["e0"] = _bf16(np.exp(etile(0, None, None)))
    e1 = np.zeros((128, 4 * 12 * N), np.float32)
    for b_, (ta, tb) in enumerate(((0, 0), (2, 2), (0, 1), (2, 3))):
        e1[:, b_ * 12 * N:(b_ + 1) * 12 * N] = etile(1, ta, tb)
    common["e1"] = _bf16(np.exp(e1))

    common["idn"] = _bf16(np.eye(128, dtype=np.float32))
    return common


def _prep_x(xb):
    return _f8(xb.T)


def kernel(x, qkv_w, qkv_b, proj_w, proj_b, rpb_table, H=None, W=None):
    x = np.asarray(x, np.float32)
    qkv_w = np.asarray(qkv_w, np.float32)
    qkv_b = np.asarray(qkv_b, np.float32)
    proj_w = np.asarray(proj_w, np.float32)
    proj_b = np.asarray(proj_b, np.float32)
    rpb_table = np.asarray(rpb_table, np.float32)

    nc = _get_nc()
    common = _host_prep(qkv_w, qkv_b, proj_w, proj_b, rpb_table)
    in_maps = []
    for b in range(B):
        m = dict(common)
        m["xin"] = _prep_x(x[b])
        in_maps.append(m)
    res = run_bass_kernel_spmd(nc, in_maps, core_ids=list(range(B)))
    out = np.stack([np.ascontiguousarray(res.results[b]["xout"].T)
                    for b in range(B)])
    return out.astype(np.float32)


# revision 34
# speedup vs baseline: 1.0510x; 1.0510x over previous
"""Swin BasicLayer (depth=2 windowed attention) Trainium2 kernel.

Sharding: data-parallel over batch B=8 across 8 NeuronCores; weights
replicated. Each core runs both depths (regular + shifted windows) over
its [12544, 384] image in [C, token] layout.

v10: fp8(e4m3)-DoubleRow QKV projections (channel-pairs packed on
partitions; x and V/QK weights quantized to fp8 with power-of-2
scaling, rescale folded into the exp scale and the vt ones column),
bf16 attention + output projection, f32 psum accumulate everywhere.
exp(S+E) computed as exp(S)*exp(E) so QK needs no psum bias preload.
Software-pipelined group loop: V-proj 2 groups ahead, normalize 1
group behind, transposes 2 groups behind.
"""
import numpy as np
import ml_dtypes

import concourse.bass as bass
import concourse.tile as tile
from concourse import bacc, mybir
from concourse.bass_utils import run_bass_kernel_spmd

f32 = mybir.dt.float32
bf16 = mybir.dt.bfloat16
f8 = mybir.dt.float8e4
AF = mybir.ActivationFunctionType
ALU = mybir.AluOpType
PM = mybir.MatmulPerfMode

B, H, W, C = 8, 112, 112, 384
NH, HD, WS = 12, 32, 7
N = WS * WS          # 49
L = H * W            # 12544
NBAND = H // WS      # 16
BAND = WS * W        # 784
HALF = BAND // 2     # 392
NWB = W // WS        # 16 windows per band
NG = NWB // 2        # 8 window-pair groups
DEPTH = 2

SCW = 16.0           # fp8 weight scale
SCX = [1.0, 32.0]    # fp8 x scale per depth (xmid stored *32)


def _rel_pos_index():
    coords = np.stack(np.meshgrid(np.arange(WS), np.arange(WS), indexing='ij')).reshape(2, -1)
    rel = (coords[:, :, None] - coords[:, None, :]).transpose(1, 2, 0).copy()
    rel[..., 0] += WS - 1
    rel[..., 1] += WS - 1
    rel[..., 0] *= 2 * WS - 1
    return rel.sum(-1)


def _masks():
    """The 4 distinct [N, N] shifted-window masks by type 2*(i==15)+(j==15)."""
    ws, shift = WS, WS // 2
    img = np.zeros((H, W), dtype=np.float32)
    slices = (slice(0, -ws), slice(-ws, -shift), slice(-shift, None))
    cnt = 0
    for hs in slices:
        for wsl in slices:
            img[hs, wsl] = cnt
            cnt += 1
    mw = img.reshape(H // ws, ws, W // ws, ws).transpose(0, 2, 1, 3).reshape(-1, ws * ws)
    diff = mw[:, None, :] - mw[:, :, None]
    mask = np.where(diff != 0, -100.0, 0.0).astype(np.float32)  # [nW, N, N]
    nwr = H // ws
    m = {}
    for ti, widx in ((0, 0), (1, nwr - 1), (2, (nwr - 1) * nwr), (3, nwr * nwr - 1)):
        m[ti] = mask[widx]
    return m


def _band_ranges(i, shift):
    r0 = (WS * i + shift) % H
    n0 = min(WS, H - r0)
    rows = [(0, r0, n0)]
    if n0 < WS:
        rows.append((n0, 0, WS - n0))
    if shift == 0:
        cols = [(0, 0, W)]
    else:
        cols = [(0, shift, W - shift), (W - shift, 0, shift)]
    return rows, cols


def _build(nbands=NBAND, ndepth=DEPTH):
    nc = bacc.Bacc("TRN2", target_bir_lowering=False, debug=False, num_devices=8)

    xin = nc.dram_tensor("xin", [C, L], f8, kind="ExternalInput")
    xmid = nc.dram_tensor("xmid", [C, L], f8)
    xout = nc.dram_tensor("xout", [C, L], f32, kind="ExternalOutput")
    dwqk8p = nc.dram_tensor("wqk8p", [DEPTH, 128, 4 * C], f8, kind="ExternalInput")
    dwqk8u = nc.dram_tensor("wqk8u", [DEPTH, 128, 2 * C], f8, kind="ExternalInput")
    dwv8p = nc.dram_tensor("wv8p", [DEPTH, 128, 2 * C], f8, kind="ExternalInput")
    dwv8u = nc.dram_tensor("wv8u", [DEPTH, 128, C], f8, kind="ExternalInput")
    dwp = nc.dram_tensor("wp", [DEPTH, C, C], bf16, kind="ExternalInput")
    dbqk = nc.dram_tensor("bqk", [DEPTH, 2 * C], f32, kind="ExternalInput")
    dbp = nc.dram_tensor("bp", [DEPTH, C], f32, kind="ExternalInput")
    de0 = nc.dram_tensor("e0", [128, 12 * N], bf16, kind="ExternalInput")
    de1 = nc.dram_tensor("e1", [128, 4 * 12 * N], bf16, kind="ExternalInput")
    didn = nc.dram_tensor("idn", [128, 128], bf16, kind="ExternalInput")

    def evac_copy(use_dve, out_ap, in_ap):
        if use_dve:
            nc.vector.tensor_copy(out_ap, in_ap)
        else:
            nc.scalar.activation(out_ap, in_ap, AF.Identity, bias=0.0)

    def evac_bias(use_dve, out_ap, in_ap, bias_ap, scale=1.0):
        if use_dve:
            if scale == 1.0:
                nc.vector.tensor_scalar_add(out_ap, in_ap, bias_ap)
            else:
                nc.vector.tensor_scalar(out_ap, in_ap, scale, bias_ap,
                                        ALU.mult, ALU.add)
        else:
            nc.scalar.activation(out_ap, in_ap, AF.Identity, bias=bias_ap,
                                 scale=scale)

    with tile.TileContext(nc) as tc:
        cpool = tc.alloc_tile_pool(name="const", bufs=1)
        p_xr = tc.alloc_tile_pool(name="xr", bufs=5)
        p_xw = tc.alloc_tile_pool(name="xw", bufs=4)
        p_qkb = tc.alloc_tile_pool(name="qkb", bufs=14)
        p_outT = tc.alloc_tile_pool(name="outT", bufs=3)
        p_yr = tc.alloc_tile_pool(name="yr", bufs=4)
        p_pt = tc.alloc_tile_pool(name="pt", bufs=4)
        p_tm = tc.alloc_tile_pool(name="ptm", bufs=3)
        p_vt = tc.alloc_tile_pool(name="vt", bufs=8)
        p_on = tc.alloc_tile_pool(name="onat", bufs=4)
        p_rt = tc.alloc_tile_pool(name="rt", bufs=3)
        ps_proj = tc.alloc_tile_pool(name="psproj", bufs=2, space="PSUM")
        ps_s = tc.alloc_tile_pool(name="pss", bufs=1, space="PSUM")
        ps_av = tc.alloc_tile_pool(name="psav", bufs=2, space="PSUM")

        # constants
        wqk8p = [cpool.tile([128, 4 * C], f8, tag=f"wqk8p{d}", name=f"wqk8p{d}")
                 for d in range(DEPTH)]
        wqk8u = [cpool.tile([128, 2 * C], f8, tag=f"wqk8u{d}", name=f"wqk8u{d}")
                 for d in range(DEPTH)]
        wv8p = [cpool.tile([128, 2 * C], f8, tag=f"wv8p{d}", name=f"wv8p{d}")
                for d in range(DEPTH)]
        wv8u = [cpool.tile([128, C], f8, tag=f"wv8u{d}", name=f"wv8u{d}")
                for d in range(DEPTH)]
        wp = [[cpool.tile([128, C], bf16, tag=f"wp{d}{cc}", name=f"wp{d}{cc}") for cc in range(3)]
              for d in range(DEPTH)]
        bqk = [cpool.tile([128, 6], f32, tag=f"bqk{d}", name=f"bqk{d}") for d in range(DEPTH)]
        bp = [cpool.tile([128, 3], f32, tag=f"bp{d}", name=f"bp{d}") for d in range(DEPTH)]
        e0 = cpool.tile([128, 12 * N], bf16, tag="e0", name="e0")
        e1 = cpool.tile([128, 4 * 12 * N], bf16, tag="e1", name="e1")
        idn = cpool.tile([128, 128], bf16, tag="idn", name="idn")
        for d in range(DEPTH):
            nc.sync.dma_start(wqk8p[d][:], dwqk8p[d])
            nc.sync.dma_start(wqk8u[d][:], dwqk8u[d])
            nc.sync.dma_start(wv8p[d][:], dwv8p[d])
            nc.sync.dma_start(wv8u[d][:], dwv8u[d])
            for cc in range(3):
                nc.sync.dma_start(wp[d][cc][:], dwp[d, cc * 128:(cc + 1) * 128, :])
            nc.sync.dma_start(bqk[d][:], dbqk[d].rearrange("(o p) -> p o", p=128))
            nc.sync.dma_start(bp[d][:], dbp[d].rearrange("(o p) -> p o", p=128))
        nc.sync.dma_start(e0[:], de0[:])
        nc.sync.dma_start(e1[:], de1[:])
        nc.sync.dma_start(idn[:], didn[:])

        for d in range(ndepth):
            shift = 0 if d == 0 else WS // 2
            xsrc = xin if d == 0 else xmid
            if d == ndepth - 1:
                ydst, ydt = xout, f32
            else:
                ydst, ydt = xmid, f8
            escale = 1.0 / (SCW * SCX[d]) ** 2
            ones_v = SCW * SCX[d]
            wqk8p_v = wqk8p[d][:].rearrange("p (i o) -> p i o", i=2)
            wv8p_v = wv8p[d][:].rearrange("p (i o) -> p i o", i=2)
            XR, XW_, QKB = {}, {}, {}

            def band_dma(bj):
                # load band: channel-pair-packed fp8 (chans 0..255 on
                # partitions as pairs) + plain fp8 (chans 256..383)
                rws, cls = _band_ranges(bj, shift)
                xrp = p_xr.tile([128, 2 * BAND], f8, tag="xrp", name="xrp")
                xru = p_xr.tile([128, BAND], f8, tag="xru", name="xru")
                xsa = xsrc[:, :]
                for (dr, sr, nr) in rws:
                    for (dc, sc, ncl) in cls:
                        for i in range(2):
                            dst = xrp[:, i * BAND:(i + 1) * BAND].rearrange(
                                "p (r c) -> p r c", r=WS)[:, dr:dr + nr, dc:dc + ncl]
                            src = bass.AP(xsa.tensor, xsa.offset + i * L + sr * W + sc,
                                          [[2 * L, 128], [W, nr], [1, ncl]])
                            nc.sync.dma_start(dst, src)
                        dstu = xru[:].rearrange("p (r c) -> p r c",
                                                r=WS)[:, dr:dr + nr, dc:dc + ncl]
                        srcu = bass.AP(xsa.tensor, xsa.offset + 256 * L + sr * W + sc,
                                       [[L, 128], [W, nr], [1, ncl]])
                        nc.sync.dma_start(dstu, srcu)
                XR[bj] = (xrp, xru)

            def band_relayout(bj):
                # window-major relayout (raster -> 49w+7r+j), fp8
                xrp, xru = XR.pop(bj)
                xwp = p_xw.tile([128, 2 * BAND], f8, tag="xwp", name="xwp")
                xwu = p_xw.tile([128, BAND], f8, tag="xwu", name="xwu")
                for i in range(2):
                    in_p = xrp[:, i * BAND:(i + 1) * BAND].rearrange(
                        "p (r w j) -> p w r j", r=WS, w=NWB, j=WS)
                    out_p = xwp[:, i * BAND:(i + 1) * BAND].rearrange(
                        "p (w r j) -> p w r j", w=NWB, r=WS, j=WS)
                    evac_copy((bj + i) % 2, out_p, in_p)
                in_u = xru[:].rearrange("p (r w j) -> p w r j", r=WS, w=NWB, j=WS)
                out_u = xwu[:].rearrange("p (w r j) -> p w r j", w=NWB, r=WS, j=WS)
                evac_copy(bj % 2, out_u, in_u)
                XW_[bj] = (xwp, xwu)

            def qkb_chain(bj, idx):
                # one (half, oc) q/k projection chain for band bj
                # (fp8 DoubleRow pass + plain fp8 pass)
                half, oc = idx // 6, idx % 6
                if idx == 0:
                    QKB[bj] = [p_qkb.tile([128, BAND], bf16, tag="qkb", name="qkb")
                               for _ in range(6)]
                xwp, xwu = XW_[bj]
                xwp_v = xwp[:].rearrange("p (i t) -> p i t", i=2)
                qkb = QKB[bj]
                ps = ps_proj.tile([128, HALF], f32, tag="psproj", name="psproj", padded_shape=[128, 512])
                nc.tensor.matmul(
                    ps[:], wqk8p_v[:, :, oc * 128:(oc + 1) * 128],
                    xwp_v[:, :, half * HALF:(half + 1) * HALF],
                    start=True, stop=False, perf_mode=PM.DoubleRow)
                nc.tensor.matmul(
                    ps[:], wqk8u[d][:, oc * 128:(oc + 1) * 128],
                    xwu[:, half * HALF:(half + 1) * HALF],
                    start=False, stop=True)
                evac_bias((oc + half) % 2,
                          qkb[oc][:, half * HALF:(half + 1) * HALF],
                          ps[:], bqk[d][:, oc:oc + 1])

            for bi in range(nbands):
                rows, cols = _band_ranges(bi, shift)
                band_dma(bi)
                band_relayout(bi)
                for idx in range(12):
                    qkb_chain(bi, idx)
                qkb = QKB.pop(bi)
                xwp, xwu = XW_[bi]
                xwp_v = xwp[:].rearrange("p (i t) -> p i t", i=2)
                # ---- attention groups, software-pipelined
                outT = p_outT.tile([128, 3 * BAND], bf16, tag="outT", name="outT")
                vts = [None] * NG
                vps = [None] * NG

                def do_vproj_mm(g):
                    vp = ps_proj.tile([128, C], f32, tag="psproj", name="psproj", padded_shape=[128, 512])
                    nc.tensor.matmul(vp[0:98, :],
                                     xwp_v[:, :, 98 * g:98 * g + 98],
                                     wv8p_v[:],
                                     start=True, stop=False, perf_mode=PM.DoubleRow)
                    nc.tensor.matmul(vp[0:98, :],
                                     xwu[:, 98 * g:98 * g + 98],
                                     wv8u[d][:],
                                     start=False, stop=True)
                    vps[g] = vp

                def do_vproj_evac(g):
                    vp = vps[g]
                    vt = p_vt.tile([128, 2 * 12 * 33], bf16, tag="vt", name="vt")
                    iv = vp[0:98, :].rearrange("p (h e) -> p h e", h=12)
                    ov = vt[0:98, 0:396].rearrange("p (h e) -> p h e", e=33)[:, :, 0:32]
                    nc.vector.tensor_copy(ov, iv)
                    nc.vector.memset(
                        vt[:, 0:396].rearrange("p (h e) -> p h e", e=33)[:, :, 32:33],
                        ones_v)
                    nc.sync.dma_start(vt[64:113, 396:792], vt[49:98, 0:396])
                    vts[g] = vt

                def ebase_sel(g):
                    if d == 0:
                        return e0, 0
                    if g < NG - 1:
                        return e1, (0 if bi < NBAND - 1 else 1) * 12 * N
                    return e1, (2 if bi < NBAND - 1 else 3) * 12 * N

                def do_transpose(g, on):
                    # transpose out [n, c] -> [c, n]; [0:113, 128] per cc
                    # (rows 49:64 are junk, skipped by the evac)
                    tp = ps_proj.tile([128, 342], bf16, tag="psproj", name="pstp", padded_shape=[128, 512])
                    for cc in range(3):
                        nc.tensor.transpose(
                            tp[:, 114 * cc:114 * cc + 113],
                            on[0:113, cc * 128:(cc + 1) * 128],
                            idn[0:113, 0:113])
                    oT = outT[:].rearrange("p f -> p f")
                    out_ap = bass.AP(oT.tensor, oT.offset + 98 * g,
                                     [oT.ap[0], [BAND, 3], [N, 2], [1, N]])
                    tpa = tp[:].rearrange("p f -> p f")
                    in_ap = bass.AP(tpa.tensor, tpa.offset,
                                    [tpa.ap[0], [114, 3], [64, 2], [1, N]])
                    evac_copy(g % 2, out_ap, in_ap)

                pts = [None] * NG
                ons = [None] * NG

                def do_exp(g, sp):
                    # exp(escale*S^T) -> bf16 tmp at 52-pitch (scalar), then
                    # pt = tmp * exp(E) elementwise (vector). exp(E) is
                    # host-precomputed; masked slots are exact zeros.
                    tm = p_tm.tile([128, 12 * 52], bf16, tag="ptm", name="ptm")
                    pt = p_pt.tile([128, 12 * 52], bf16, tag="pt", name="pt")
                    spa = sp[:]
                    tm_ = tm[:].rearrange("p f -> p f")
                    po_ = pt[:].rearrange("p f -> p f")
                    exp_in = bass.AP(spa.tensor, spa.offset,
                                     [spa.ap[0], [512, 4], [N, 3], [1, N]])
                    exp_out = bass.AP(tm_.tensor, tm_.offset,
                                      [tm_.ap[0], [52, 4], [4 * 52, 3], [1, N]])
                    nc.scalar.activation(exp_out, exp_in, AF.Exp, scale=escale)
                    et, ebase = ebase_sel(g)
                    ea = et[:].rearrange("p f -> p f")
                    e_in = bass.AP(ea.tensor, ea.offset + ebase,
                                   [ea.ap[0], [N, 4], [4 * N, 3], [1, N]])
                    t_in = bass.AP(tm_.tensor, tm_.offset,
                                   [tm_.ap[0], [52, 4], [4 * 52, 3], [1, N]])
                    p_out = bass.AP(po_.tensor, po_.offset,
                                    [po_.ap[0], [52, 4], [4 * 52, 3], [1, N]])
                    nc.vector.tensor_tensor(p_out, t_in, e_in, ALU.mult)
                    pts[g] = pt

                avs = [None] * NG

                def do_av_mm(g):
                    # AV (+ fused row-sums via ones column of vt); h-outer so
                    # consecutive matmuls alternate PE tiles (LDW pull-ahead)
                    pt, vt = pts[g], vts[g]
                    av = ps_av.tile([128, 12 * 33], f32, tag="psav", name="psav", padded_shape=[128, 512])
                    for h in range(NH):
                        for s in range(2):
                            nc.tensor.matmul(
                                av[64 * s:64 * s + 49, 33 * h:33 * h + 33],
                                pt[64 * s:64 * s + 49, 52 * h:52 * h + N],
                                vt[64 * s:64 * s + 49,
                                   396 * s + 33 * h:396 * s + 33 * h + 33],
                                start=True, stop=True,
                                tile_position=(64 * s, 64 * s))
                    avs[g] = av

                def do_norm(g):
                    av = avs[g]
                    rt = p_rt.tile([128, 12], f32, tag="rt", name="rt")
                    nc.vector.reciprocal(
                        rt[:].rearrange("p (h e) -> p h e", e=1),
                        av[:].rearrange("p (h e) -> p h e", e=33)[:, :, 32:33])
                    on = p_on.tile([128, C], bf16, tag="onat", name="onat")
                    rap = rt[:]
                    rbc = bass.AP(rap.tensor, rap.offset, [rap.ap[0], [1, 12], [0, 32]])
                    nc.vector.tensor_tensor(
                        on[:].rearrange("p (h e) -> p h e", e=32),
                        av[:].rearrange("p (h e) -> p h e", e=33)[:, :, 0:32],
                        rbc, ALU.mult)
                    ons[g] = on

                do_vproj_mm(0)
                do_vproj_evac(0)
                do_vproj_mm(1)
                do_vproj_evac(1)
                for g in range(NG):
                    sp = ps_s.tile([128, 2048], f32, tag="pss", name="pss")
                    for s in range(2):
                        w = 2 * g + s
                        for h in range(NH):
                            po = 32 * (h % 4)
                            kT = qkb[3 + h // 4][po:po + 32, N * w:N * w + N]
                            qT = qkb[h // 4][po:po + 32, N * w:N * w + N]
                            out = sp[64 * s:64 * s + 49,
                                     512 * (h % 4) + N * (h // 4):
                                     512 * (h % 4) + N * (h // 4) + N]
                            nc.tensor.matmul(out, kT, qT, start=True, stop=True,
                                             tile_position=(po, 64 * s))
                    do_exp(g, sp)
                    if g + 2 < NG:
                        do_vproj_mm(g + 2)
                    if g >= 2:
                        do_transpose(g - 2, ons[g - 2])
                    do_av_mm(g)
                    if g >= 1:
                        do_norm(g - 1)
                    if g + 2 < NG:
                        do_vproj_evac(g + 2)
                do_norm(NG - 1)
                do_transpose(NG - 2, ons[NG - 2])
                do_transpose(NG - 1, ons[NG - 1])
                # ---- output projection (+ window-major -> raster relayout)
                yr = [p_yr.tile([128, BAND], ydt, tag="yr", name="yr") for _ in range(3)]
                ysc = SCX[1] if d == 0 and ndepth > 1 else 1.0
                for half in range(2):
                    for oc in range(3):
                        ps = ps_proj.tile([128, HALF], f32, tag="psproj", name="psproj", padded_shape=[128, 512])
                        for cc in range(3):
                            nc.tensor.matmul(
                                ps[:], wp[d][cc][:, oc * 128:(oc + 1) * 128],
                                outT[:, cc * BAND + half * HALF:
                                     cc * BAND + (half + 1) * HALF],
                                start=(cc == 0), stop=(cc == 2))
                        in_ap = ps[:].rearrange("p (w r j) -> p w r j",
                                                w=NG, r=WS, j=WS)
                        out_ap = yr[oc][:].rearrange(
                            "p (r w j) -> p w r j", r=WS, w=NWB,
                            j=WS)[:, NG * half:NG * half + NG, :, :]
                        evac_bias((oc + half) % 2, out_ap, in_ap,
                                  bp[d][:, oc:oc + 1], scale=ysc)
                # ---- store band
                for oc in range(3):
                    yr3 = yr[oc][:].rearrange("p (r c) -> p r c", r=WS)
                    dst3 = ydst[oc * 128:(oc + 1) * 128, :].rearrange(
                        "p (r c) -> p r c", r=H)
                    for (dr, sr, nr) in rows:
                        for (dc, sc, ncl) in cols:
                            nc.sync.dma_start(dst3[:, sr:sr + nr, sc:sc + ncl],
                                              yr3[:, dr:dr + nr, dc:dc + ncl])
                del XW_[bi]
            if d == 0 and ndepth > 1:
                tc.strict_bb_all_engine_barrier()

        for p in (ps_av, ps_s, ps_proj, p_rt, p_on, p_vt,
                  p_tm, p_pt, p_yr, p_outT, p_qkb, p_xw, p_xr, cpool):
            p.release()

    nc.compile()
    return nc


_NC = None


def _get_nc():
    global _NC
    if _NC is None:
        _NC = _build()
    return _NC


def _bf16(x):
    return np.ascontiguousarray(x).astype(ml_dtypes.bfloat16)


def _f8(x):
    return np.ascontiguousarray(x).astype(mybir.dt.np(f8))


def _pair_pack(w):
    """[256, M] -> [128, 2M], i-outer: col i*M+m = w[2p+i, m]."""
    M = w.shape[1]
    return w.reshape(128, 2 * M)


def _host_prep(qkv_w, qkv_b, proj_w, proj_b, rpb_table):
    scale = HD ** -0.5
    rpi = _rel_pos_index()
    masks = _masks()
    common = {}
    wqk8p = np.zeros((DEPTH, 128, 4 * C), np.float32)
    wqk8u = np.zeros((DEPTH, 128, 2 * C), np.float32)
    wv8p = np.zeros((DEPTH, 128, 2 * C), np.float32)
    wv8u = np.zeros((DEPTH, 128, C), np.float32)
    wpp = np.zeros((DEPTH, C, C), np.float32)
    bqk = np.zeros((DEPTH, 2 * C), np.float32)
    bpp = np.zeros((DEPTH, C), np.float32)
    for d in range(DEPTH):
        wq = qkv_w[d][:2 * C].T.copy()        # [C, 2C] (q then k)
        wq[:, :C] *= scale
        wq *= SCW
        wqk8p[d] = _pair_pack(wq[:256])
        wqk8u[d] = wq[256:]
        wvv = qkv_w[d][2 * C:].T * SCW        # [C, C]
        wv8p[d] = _pair_pack(wvv[:256])
        wv8u[d] = wvv[256:]
        wpp[d] = proj_w[d].T
        bq = qkv_b[d][:2 * C].copy()
        bq[:C] *= scale
        bqk[d] = bq * (SCW * SCX[d])
        bv = qkv_b[d][2 * C:]
        bpp[d] = proj_b[d] + proj_w[d] @ bv
    bpp[0] *= SCX[1]   # depth-0 output stored as fp8 * SCX[1]
    common["wqk8p"] = _f8(wqk8p)
    common["wqk8u"] = _f8(wqk8u)
    common["wv8p"] = _f8(wv8p)
    common["wv8u"] = _f8(wv8u)
    common["wp"] = _bf16(wpp)
    common["bqk"] = bqk
    common["bp"] = bpp

    # E tiles: rows 0-48 -> m, rows 64-112 -> m-64; value exp(bias[h,n,m]+mask[n,m])
    def etile(d, type_a, type_b):
        bias = rpb_table[d][rpi]              # [N, N, NH]
        t = np.zeros((128, 12 * N), np.float32)
        for s, ty in ((0, type_a), (1, type_b)):
            bm = bias + (masks[ty][:, :, None] if ty is not None else 0.0)
            ev = bm.transpose(2, 1, 0)   # [NH, m, n] (log domain)
            blk = ev.transpose(1, 0, 2).reshape(N, 12 * N)  # row m, col h*N+n
            t[64 * s:64 * s + N, :] = blk
        return t

    common["e0"] = _bf16(np.exp(etile(0, None, None)))
    e1 = np.zeros((128, 4 * 12 * N), np.float32)
    for b_, (ta, tb) in enumerate(((0, 0), (2, 2), (0, 1), (2, 3))):
        e1[:, b_ * 12 * N:(b_ + 1) * 12 * N] = etile(1, ta, tb)
    common["e1"] = _bf16(np.exp(e1))

    common["idn"] = _bf16(np.eye(128, dtype=np.float32))
    return common


def _prep_x(xb):
    return _f8(xb.T)


def kernel(x, qkv_w, qkv_b, proj_w, proj_b, rpb_table, H=None, W=None):
    x = np.asarray(x, np.float32)
    qkv_w = np.asarray(qkv_w, np.float32)
    qkv_b = np.asarray(qkv_b, np.float32)
    proj_w = np.asarray(proj_w, np.float32)
    proj_b = np.asarray(proj_b, np.float32)
    rpb_table = np.asarray(rpb_table, np.float32)

    nc = _get_nc()
    common = _host_prep(qkv_w, qkv_b, proj_w, proj_b, rpb_table)
    in_maps = []
    for b in range(B):
        m = dict(common)
        m["xin"] = _prep_x(x[b])
        in_maps.append(m)
    res = run_bass_kernel_spmd(nc, in_maps, core_ids=list(range(B)))
    out = np.stack([np.ascontiguousarray(res.results[b]["xout"].T)
                    for b in range(B)])
    return out.astype(np.float32)
